# revision 1
# baseline (speedup 1.0000x reference)
"""Chamfer rate-distortion loss on 8 TRN2 NeuronCores.

Layout: 8 cores = 4 batches x 2 chamfer directions. Each core computes, for
its (batch, direction), the per-point nearest-neighbor squared distance of
8192 query points X against 8192 reference points Y.

Device algorithm per core:
  - X and Y are pre-sorted (host) along coordinate AXIS.
  - matmul trick (fp16 hi/lo split, K=11, full PE rate, ~1e-5 abs precision):
    PSUM[m,p] = SCALE^2*(|y_p|^2 - 2 x_m.y_p) = SCALE^2*(D[m,p] - |x_m|^2).
    DVE reduce_min along the free axis gives the per-query band minimum
    (|x|^2 added and rescaled on host).
  - 256 col-tiled sub-chunks of 32 sorted queries (4 per 128-partition PSUM
    block via tile_position) each scan a BAND-wide window of sorted Y around
    their own sorted position; edges padded with duplicates of the extreme
    real points (which can never lower a min below the true min).
  - 16 overflow windows scan the FULL Y for up to 128 "hard" points whose
    nearest neighbor may fall outside their band (selected on host with a
    conservative Morton-neighbor upper bound).

Exactness: for a query x, every Y outside its band differs from x along the
sort axis by at least gap(x), so any excluded point has D >= gap^2.  Host
verifies band_min_D <= gap^2 per point (sound, data-independent); the rare
unverified points are recomputed exactly on host (on expected data: none).
"""

import os

import numpy as np

B, M, P = 4, 8192, 8192
AXIS = 2
SUB = 32             # sub-chunk: 32 sorted queries share one band
BAND = 416           # uniform band width (rt cols) per sub-chunk
PAD = 192            # edge-dup pad = guaranteed halfwidth; band = [32u, 32u+416)
NBLK = 64            # blocks of 128 queries = 4 col-tiled sub-chunks
GRP = 4              # blocks per PSUM tile / per reduce op
OBAND = 512          # overflow window width
NOVER = 16           # overflow windows of 512: full 8192 scan
NOUT = NBLK + NOVER
WT_W = M + 128       # sorted queries + 128 overflow slots
RT_W = P + 2 * PAD   # pad + sorted refs + pad
KROWS = 11           # fp16 hi/lo decomposition rows (see _prep_core)
SCALE = 32.0         # coordinate pre-scale; device min is SCALE^2 * real
LMBDA = 5.0

_CACHE = {}
LAST_RESULTS = None


def _build_bass():
    import concourse.tile as tile
    from concourse import bacc, mybir

    nc = bacc.Bacc(None, target_bir_lowering=False, debug=False)
    f32 = mybir.dt.float32
    f16 = mybir.dt.float16

    wr_d = nc.dram_tensor("wr", [KROWS, WT_W + RT_W], f16, kind="ExternalInput")
    out_d = nc.dram_tensor("out", [128, NOUT], f32, kind="ExternalOutput")

    with tile.TileContext(nc) as tc:
        with (
            tc.tile_pool(name="const", bufs=1) as cpool,
            tc.tile_pool(name="outp", bufs=1) as opool,
            tc.tile_pool(name="psum", bufs=2, space="PSUM") as ppool,
        ):
            wr = cpool.tile([KROWS, WT_W + RT_W], f16)
            # head pieces (first ~48 sub-chunks' weights + bands) land first
            # so the PE can start while the bulk of the input streams in
            HW_, HR_ = 1536, 2048
            nc.sync.dma_start(wr[:, 0:HW_], wr_d[:, 0:HW_])
            nc.sync.dma_start(wr[:, WT_W:WT_W + HR_], wr_d[:, WT_W:WT_W + HR_])
            nc.sync.dma_start(wr[:, HW_:WT_W], wr_d[:, HW_:WT_W])
            nc.sync.dma_start(wr[:, WT_W + HR_:], wr_d[:, WT_W + HR_:])
            outt = opool.tile([128, NOUT], f32)

            for k in range(NBLK // GRP):
                # group stride 512 keeps every matmul output bank-aligned;
                # only cols [0, BAND) are written and reduced
                ps = ppool.tile([128, GRP, 512], f32, tag="ps")
                for g in range(GRP):
                    for s in range(4):
                        u = 4 * (GRP * k + g) + s   # global sub-chunk index
                        nc.tensor.matmul(
                            ps[32 * s:32 * s + 32, g, 0:BAND],
                            wr[:, SUB * u:SUB * u + SUB],
                            wr[:, WT_W + SUB * u:WT_W + SUB * u + BAND],
                            start=True, stop=True,
                            tile_position=(0, 32 * s),
                        )
                nc.vector.tensor_reduce(outt[:, GRP * k:GRP * (k + 1)],
                                        ps[:, :, 0:BAND],
                                        axis=mybir.AxisListType.X,
                                        op=mybir.AluOpType.min)

            for k in range(NOVER // GRP):
                ps = ppool.tile([128, GRP, OBAND], f32, tag="ps")
                for g in range(GRP):
                    j = GRP * k + g   # overflow window index
                    rcol = PAD + OBAND * j
                    nc.tensor.matmul(ps[:, g, :], wr[:, M:M + 128],
                                     wr[:, WT_W + rcol:WT_W + rcol + OBAND],
                                     start=True, stop=True)
                nc.vector.tensor_reduce(
                    outt[:, NBLK + GRP * k:NBLK + GRP * (k + 1)], ps[:],
                    axis=mybir.AxisListType.X, op=mybir.AluOpType.min)

            nc.sync.dma_start(out_d[:], outt[:])
    nc.compile()
    return nc


def _morton_key(pts):
    rng = pts.max(0) - pts.min(0)
    q = ((pts - pts.min(0)) / (rng + 1e-9) * 1023).astype(np.uint64)

    def spread(x):
        x = x & np.uint64(0x3FF)
        x = (x | (x << np.uint64(16))) & np.uint64(0x30000FF)
        x = (x | (x << np.uint64(8))) & np.uint64(0x300F00F)
        x = (x | (x << np.uint64(4))) & np.uint64(0x30C30C3)
        x = (x | (x << np.uint64(2))) & np.uint64(0x9249249)
        return x

    return (spread(q[:, 0]) | (spread(q[:, 1]) << np.uint64(1))
            | (spread(q[:, 2]) << np.uint64(2)))


def _prep_core(X, Y):
    """Host prep for one (batch, direction): returns in_map plus the metadata
    needed to verify and assemble the result."""
    xo = np.argsort(X[:, AXIS], kind="stable")
    yo = np.argsort(Y[:, AXIS], kind="stable")
    Xs = X[xo]
    Ys = Y[yo]
    X2 = (Xs.astype(np.float64) ** 2).sum(1)
    Y2 = (Ys.astype(np.float64) ** 2).sum(1)
    zx = Xs[:, AXIS].astype(np.float64)
    zy = Ys[:, AXIS].astype(np.float64)

    # gap to nearest excluded Y along the sort axis, per query
    i = np.arange(M)
    c = i // SUB
    lo_pos = SUB * c - PAD          # first included Y position
    hi_pos = SUB * c + (BAND - PAD)  # first excluded upper position
    gap = np.full(M, np.inf)
    has_lo = lo_pos > 0
    gap[has_lo] = zx[has_lo] - zy[lo_pos[has_lo] - 1]
    has_hi = hi_pos < P
    gap[has_hi] = np.minimum(gap[has_hi], zy[hi_pos[has_hi]] - zx[has_hi])
    gap = np.maximum(gap, 0.0)

    # conservative NN-distance upper bound via Morton-order neighbors
    allpts = np.concatenate([Xs, Ys]).astype(np.float64)
    mk = _morton_key(allpts)
    inv = np.empty(2 * M, dtype=np.int64)
    inv[np.argsort(mk, kind="stable")] = np.arange(2 * M)
    y_rank = inv[M:]
    order_y = np.argsort(y_rank, kind="stable")
    sorted_ranks = y_rank[order_y]
    K = 16
    idx = np.searchsorted(sorted_ranks, inv[:M])
    cand = np.clip(idx[:, None] + np.arange(-K, K)[None, :], 0, M - 1)
    cands = order_y[cand]
    d2 = ((Xs[:, None, :].astype(np.float64) - Ys[cands].astype(np.float64)) ** 2).sum(-1)
    d_cap2 = d2.min(1)

    hard = np.flatnonzero(~(d_cap2 <= (gap * gap) * 0.98))
    if len(hard) > 128:
        score = np.sqrt(d_cap2[hard]) - gap[hard]
        hard = hard[np.argsort(-score)[:128]]
    over_idx = np.full(128, hard[0] if len(hard) else 0, dtype=np.int64)
    over_idx[:len(hard)] = hard

    # fp16 hi/lo decomposition of SCALE*X and SCALE*Y; device computes
    # SCALE^2 * (|y|^2 - 2 x.y) in fp32 PSUM via K=11 contraction rows:
    #   r0-2: -2*a_d * c_d     r3-5: -2*a_d * e_d     r6-8: -2*b_d * c_d
    #   r9:   1 * w_hi         r10:  1 * w_lo
    # where a+b ~ SCALE*x, c+e ~ SCALE*y, w_hi+w_lo ~ |SCALE*y|^2.
    Xss = (SCALE * Xs).astype(np.float64)
    Yss = (SCALE * Ys).astype(np.float64)
    a = Xss.astype(np.float16)
    bb = (Xss - a.astype(np.float64)).astype(np.float16)
    c = Yss.astype(np.float16)
    e = (Yss - c.astype(np.float64)).astype(np.float16)
    w = (Yss ** 2).sum(1)
    wh = w.astype(np.float16)
    wl = (w - wh.astype(np.float64)).astype(np.float16)

    wr = np.empty((KROWS, WT_W + RT_W), dtype=np.float16)
    wt = wr[:, :WT_W]
    rt = wr[:, WT_W:]

    na = (-2.0 * a.astype(np.float64)).astype(np.float16)  # exact: x2 of fp16
    nb = (-2.0 * bb.astype(np.float64)).astype(np.float16)
    wt[0:3, :M] = na.T
    wt[3:6, :M] = na.T
    wt[6:9, :M] = nb.T
    wt[9:11, :M] = 1.0
    wt[0:3, M:] = na[over_idx].T
    wt[3:6, M:] = na[over_idx].T
    wt[6:9, M:] = nb[over_idx].T
    wt[9:11, M:] = 1.0

    ccT = c.T
    eeT = e.T
    # edge-duplicate padding: repeats of the first/last sorted reference
    # point — real candidates, can never lower a min below the true min.
    for cols, sl in ((slice(0, PAD), 0), (slice(PAD + P, RT_W), P - 1)):
        rt[0:3, cols] = ccT[:, sl:sl + 1]
        rt[3:6, cols] = eeT[:, sl:sl + 1]
        rt[6:9, cols] = ccT[:, sl:sl + 1]
        rt[9, cols] = wh[sl]
        rt[10, cols] = wl[sl]
    rt[0:3, PAD:PAD + P] = ccT
    rt[3:6, PAD:PAD + P] = eeT
    rt[6:9, PAD:PAD + P] = ccT
    rt[9, PAD:PAD + P] = wh
    rt[10, PAD:PAD + P] = wl

    return {"wr": wr}, {
        "Xs": Xs.astype(np.float64), "Ys": Ys.astype(np.float64),
        "X2": X2, "Y2": Y2, "gap": gap, "hard": hard, "over_idx": over_idx,
    }


def _post_core(out, meta):
    """Combine device output into sum over queries of min-D (float64)."""
    inv_s2 = 1.0 / (SCALE * SCALE)
    band_min = out[:, :NBLK].T.reshape(M).astype(np.float64) * inv_s2
    dmin = band_min + meta["X2"]

    over_min = out[:, NBLK:].min(axis=1).astype(np.float64) * inv_s2
    over_d = over_min + meta["X2"][meta["over_idx"]]
    nhard = len(meta["hard"])
    if nhard:
        dmin[meta["hard"]] = over_d[:nhard]

    # soundness check for band-only points (device fp32 margin included)
    g2 = meta["gap"] * meta["gap"]
    ok = dmin <= g2 - 1e-3 - 1e-3 * np.abs(dmin)
    ok[meta["hard"]] = True
    bad = np.flatnonzero(~ok)
    if len(bad):
        Xb = meta["Xs"][bad]
        db = (meta["Y2"][None, :] - 2.0 * (Xb @ meta["Ys"].T)).min(axis=1)
        dmin[bad] = db + meta["X2"][bad]
    return dmin.sum()


def _install_axon_profile_hook():
    """Make trace=True work under axon when the image's antenv lacks
    axon_hooks: inject a shim module wired to the ctypes NTFF driver."""
    import sys
    import types
    try:
        from antenv.axon_hooks import get_axon_ntff_profile_hook  # noqa: F401
        return
    except ImportError:
        pass
    try:
        import antenv
        from trn_agent_boot.trn_boot import _ntff_profile_via_ctypes
        hook = _ntff_profile_via_ctypes("/opt/axon/libaxon_pjrt.so")
    except Exception:
        hook = None
    mod = types.ModuleType("antenv.axon_hooks")
    state = {"h": hook}
    mod.get_axon_ntff_profile_hook = lambda: state["h"]
    mod.set_axon_ntff_profile_hook = lambda h: state.__setitem__("h", h)
    sys.modules["antenv.axon_hooks"] = mod
    try:
        antenv.axon_hooks = mod
    except Exception:
        pass


def kernel(x_hat, points, likelihoods):
    from concourse.bass_utils import run_bass_kernel_spmd
    global LAST_RESULTS

    trace = bool(int(os.environ.get("CHAMFER_TRACE", "0")))
    if trace:
        _install_axon_profile_hook()

    if "nc" not in _CACHE:
        _CACHE["nc"] = _build_bass()
    nc = _CACHE["nc"]

    in_maps, metas = [], []
    for core in range(8):
        b, d = core // 2, core % 2
        X = x_hat[b] if d == 0 else points[b]
        Y = points[b] if d == 0 else x_hat[b]
        m, meta = _prep_core(np.asarray(X), np.asarray(Y))
        in_maps.append(m)
        metas.append(meta)

    res = run_bass_kernel_spmd(
        nc, in_maps, core_ids=list(range(8)), trace=trace,
    )
    LAST_RESULTS = res

    sums = [_post_core(res.results[c]["out"], metas[c]) for c in range(8)]
    cham_x = sum(sums[c] for c in range(8) if c % 2 == 0) / (B * M)
    cham_y = sum(sums[c] for c in range(8) if c % 2 == 1) / (B * P)
    rec = cham_x + cham_y

    lik = np.asarray(likelihoods, dtype=np.float64)
    bpp = np.log2(lik).sum() / (-(B * P))

    loss = bpp + LMBDA * rec
    return np.array([loss, bpp, rec], dtype=np.float32)



# revision 18
# speedup vs baseline: 1.0710x; 1.0710x over previous
"""Chamfer rate-distortion loss on 8 TRN2 NeuronCores.

Layout: 8 cores = 4 batches x 2 chamfer directions. Each core computes, for
its (batch, direction), the per-point nearest-neighbor squared distance of
8192 query points X against 8192 reference points Y.

Device algorithm per core (v2):
  - X and Y pre-sorted (host) along coordinate AXIS. 64 blocks of S=128
    consecutive sorted queries each scan one shared band of W sorted refs
    (guaranteed halfwidth H ranks beyond the block edges, edge-dup padded).
  - fp16 hi/lo matmul trick (K=13 rows): PSUM[m,p] = SCALE^2*|x~_m - y~_p|^2
    (both squared norms folded in as extra contraction rows) so values near
    the min are tiny and survive fp16 lane outputs exactly enough.
  - Table is stored in 4 partition-group chunks (PE row-tiles 0/32/64/96) so
    the input DMA spans 52 SBUF partitions instead of 13.
  - Per-block min over the band is computed by one tensor_tensor_reduce
    (elementwise min of the band halves + min-reduce, single instruction),
    load-balanced across the DVE and GpSimd engines, with the Activation
    engine converting some blocks to fp16 for a fast DVE reduce.
  - 16 overflow matmuls scan the FULL Y for up to 128 "hard" queries whose
    NN may fall outside their band (host-selected via Morton-neighbor bound).

Exactness: host verifies band_min <= gap^2 per query (gap = sort-axis
distance to the nearest excluded ref); unverified queries are recomputed
exactly on host (expected: none beyond the hard set).
"""

import os

import numpy as np

B, M, P = 4, 8192, 8192
AXIS = 2
S = 128              # queries per block
NBLK = M // S        # 64
W = 448              # band width per block
H = (W - S) // 2     # 160 guaranteed halfwidth
HPAD = 192           # table pad (max supported halfwidth)
NOUT = NBLK          # 64 output cols
KROWS = 13
SCALE = 16.0
LMBDA = 5.0

NGRP = 4             # partition groups (PE row-tiles)
BPG = NBLK // NGRP   # 16 blocks per group
WT_CW = S * BPG                # 2048 weight cols per chunk
RT_CW = P // NGRP + 2 * HPAD   # 2432 ref cols per chunk
CHUNK_W = WT_CW + RT_CW        # 4480

# blocks are processed round-robin across the 4 PE row-tile groups so each
# LDWEIGHTS can overlap the previous matmul's stream; outt col = process idx
ORDER = [(i % 4) * BPG + i // 4 for i in range(NBLK)]

ROWTILES = True      # 4 PE row-tile groups (fast DMA); False = single group

# Each band is drained from PSUM by two engines: the Act engine converts
# FCONV cols to fp16 SBUF (DVE re-reduces those 4x fast), DVE min-reduces
# the remaining FDIR cols straight from PSUM.  Host combines the two
# partial mins.  Widths chosen to balance Act vs DVE busy time.
FCONV = 312          # cols converted by Act per block
FDIR = W - FCONV     # cols reduced directly by DVE per block
UB = 4               # blocks per psum tile / lane unit

_CACHE = {}
LAST_RESULTS = None


def _build_bass():
    import concourse.tile as tile
    from concourse import bacc, mybir

    nc = bacc.Bacc(None, target_bir_lowering=False, debug=False)
    f32 = mybir.dt.float32
    f16 = mybir.dt.float16
    AX = mybir.AxisListType
    OP = mybir.AluOpType

    wr_d = nc.dram_tensor("wr", [KROWS, NGRP, CHUNK_W], f16, kind="ExternalInput")
    out_d = nc.dram_tensor("out", [128, 2 * NOUT], f32, kind="ExternalOutput")

    def base(j):
        return 32 * j if ROWTILES else 0

    def col(j, x):
        return x if ROWTILES else j * CHUNK_W + x

    with tile.TileContext(nc) as tc:
        with (
            tc.tile_pool(name="const", bufs=1) as cpool,
            tc.tile_pool(name="outp", bufs=1) as opool,
            tc.tile_pool(name="scr", bufs=4) as spool,
            tc.tile_pool(name="psum", bufs=2, space="PSUM") as ppool,
        ):
            wr = cpool.tile([128, CHUNK_W if ROWTILES else NGRP * CHUNK_W], f16)
            # chunk j lives at partitions 32j..32j+KROWS; load every group's
            # head first (round-robin block order touches all groups at once)
            dma_engs = [nc.sync, nc.scalar, nc.gpsimd, nc.sync]
            for j in range(NGRP):
                eng = dma_engs[j]
                v = wr[base(j):base(j) + KROWS, :]
                eng.dma_start(v[:, col(j, 0):col(j, 256)],
                              wr_d[:, j, 0:256])
                eng.dma_start(v[:, col(j, WT_CW):col(j, WT_CW + 768)],
                              wr_d[:, j, WT_CW:WT_CW + 768])
            for j in range(NGRP):
                eng = dma_engs[j]
                v = wr[base(j):base(j) + KROWS, :]
                eng.dma_start(v[:, col(j, 256):col(j, WT_CW)],
                              wr_d[:, j, 256:WT_CW])
                eng.dma_start(v[:, col(j, WT_CW + 768):col(j, CHUNK_W)],
                              wr_d[:, j, WT_CW + 768:CHUNK_W])
            outt = opool.tile([128, 2 * NOUT], f32)

            def lane(ps, cols, u):
                """Drain psum tile ps[:, 0:UB, 0:W] into outt[:, cols:...]."""
                cnv = spool.tile([128, UB, FCONV], f16, tag=f"cnv{u % 4}")
                nc.scalar.activation(
                    cnv[:], ps[:, :, FDIR:W],
                    mybir.ActivationFunctionType.Copy,
                )
                nc.vector.tensor_reduce(
                    outt[:, cols:cols + UB], ps[:, :, 0:FDIR],
                    axis=AX.X, op=OP.min)
                nc.vector.tensor_reduce(
                    outt[:, NOUT + cols:NOUT + cols + UB], cnv[:],
                    axis=AX.X, op=OP.min)

            for u in range(NBLK // UB):
                ps = ppool.tile([128, UB, 512], f32, tag="ps")
                for g in range(UB):
                    c = ORDER[UB * u + g]
                    j, cc = c // BPG, c % BPG
                    nc.tensor.matmul(
                        ps[:, g, 0:W],
                        wr[base(j):base(j) + KROWS,
                           col(j, S * cc):col(j, S * cc + S)],
                        wr[base(j):base(j) + KROWS,
                           col(j, WT_CW + S * cc + (HPAD - H)):
                           col(j, WT_CW + S * cc + (HPAD - H) + W)],
                        start=True, stop=True,
                        tile_position=(base(j), 0),
                    )
                lane(ps, UB * u, u)

            nc.sync.dma_start(out_d[:], outt[:])
    nc.compile()
    return nc


def _morton_key(pts):
    rng = pts.max(0) - pts.min(0)
    q = ((pts - pts.min(0)) / (rng + 1e-9) * 1023).astype(np.uint64)

    def spread(x):
        x = x & np.uint64(0x3FF)
        x = (x | (x << np.uint64(16))) & np.uint64(0x30000FF)
        x = (x | (x << np.uint64(8))) & np.uint64(0x300F00F)
        x = (x | (x << np.uint64(4))) & np.uint64(0x30C30C3)
        x = (x | (x << np.uint64(2))) & np.uint64(0x9249249)
        return x

    return (spread(q[:, 0]) | (spread(q[:, 1]) << np.uint64(1))
            | (spread(q[:, 2]) << np.uint64(2)))


def _prep_core(X, Y):
    """Host prep for one (batch, direction): returns in_map plus metadata."""
    xo = np.argsort(X[:, AXIS], kind="stable")
    yo = np.argsort(Y[:, AXIS], kind="stable")
    Xs = X[xo]
    Ys = Y[yo]
    zx = Xs[:, AXIS].astype(np.float64)
    zy = Ys[:, AXIS].astype(np.float64)

    # gap to nearest excluded Y along the sort axis, per query
    i = np.arange(M)
    c = i // S
    lo_pos = S * c - H
    hi_pos = S * c + S + H
    gap = np.full(M, np.inf)
    has_lo = lo_pos > 0
    gap[has_lo] = zx[has_lo] - zy[lo_pos[has_lo] - 1]
    has_hi = hi_pos < P
    gap[has_hi] = np.minimum(gap[has_hi], zy[hi_pos[has_hi]] - zx[has_hi])
    gap = np.maximum(gap, 0.0)

    # conservative NN-distance upper bound via Morton-order neighbors
    allpts = np.concatenate([Xs, Ys]).astype(np.float64)
    mk = _morton_key(allpts)
    inv = np.empty(2 * M, dtype=np.int64)
    inv[np.argsort(mk, kind="stable")] = np.arange(2 * M)
    y_rank = inv[M:]
    order_y = np.argsort(y_rank, kind="stable")
    sorted_ranks = y_rank[order_y]
    K = 32
    idx = np.searchsorted(sorted_ranks, inv[:M])
    cand = np.clip(idx[:, None] + np.arange(-K, K)[None, :], 0, M - 1)
    cands = order_y[cand]
    Xs32 = Xs.astype(np.float32)
    Ys32 = Ys.astype(np.float32)
    d2 = ((Xs32[:, None, :] - Ys32[cands]) ** 2).sum(-1).astype(np.float64)
    d_cap2 = d2.min(1) * 1.01 + 1e-9

    hard = np.flatnonzero(~(d_cap2 <= (gap * gap) * 0.98))

    # fp16 hi/lo decomposition of SCALE*X and SCALE*Y; device computes
    # SCALE^2 * |x~ - y~|^2 in fp32 PSUM via K=13 contraction rows:
    #   r0-2: -2*a_d * c_d     r3-5: -2*a_d * e_d     r6-8: -2*b_d * c_d
    #   r9:   1 * w_hi         r10:  1 * w_lo  (w = |SCALE*y~|^2)
    #   r11:  u_hi * 1         r12:  u_lo * 1  (u = |SCALE*x~|^2)
    Xss = (SCALE * Xs).astype(np.float64)
    Yss = (SCALE * Ys).astype(np.float64)
    a = Xss.astype(np.float16)
    bb = (Xss - a.astype(np.float64)).astype(np.float16)
    cc_ = Yss.astype(np.float16)
    e = (Yss - cc_.astype(np.float64)).astype(np.float16)
    xt = a.astype(np.float64) + bb.astype(np.float64)   # SCALE*x~
    yt = cc_.astype(np.float64) + e.astype(np.float64)  # SCALE*y~
    w = (yt ** 2).sum(1)
    wh = w.astype(np.float16)
    wl = (w - wh.astype(np.float64)).astype(np.float16)
    u = (xt ** 2).sum(1)
    uh = u.astype(np.float16)
    ul = (u - uh.astype(np.float64)).astype(np.float16)

    na = (-2.0 * a.astype(np.float64)).astype(np.float16)  # exact: x2 of fp16
    nb = (-2.0 * bb.astype(np.float64)).astype(np.float16)

    # full weight table [13, M] and padded ref table [13, P + 2*HPAD]
    wt = np.empty((KROWS, M), dtype=np.float16)
    wt[0:3, :] = na.T
    wt[3:6, :] = na.T
    wt[6:9, :] = nb.T
    wt[9:11, :] = 1.0
    wt[11, :] = uh
    wt[12, :] = ul

    RT_W = P + 2 * HPAD
    rt = np.empty((KROWS, RT_W), dtype=np.float16)
    ccT = cc_.T
    eeT = e.T
    for cols, sl in ((slice(0, HPAD), 0), (slice(HPAD + P, RT_W), P - 1)):
        rt[0:3, cols] = ccT[:, sl:sl + 1]
        rt[3:6, cols] = eeT[:, sl:sl + 1]
        rt[6:9, cols] = ccT[:, sl:sl + 1]
        rt[9, cols] = wh[sl]
        rt[10, cols] = wl[sl]
    rt[0:3, HPAD:HPAD + P] = ccT
    rt[3:6, HPAD:HPAD + P] = eeT
    rt[6:9, HPAD:HPAD + P] = ccT
    rt[9, HPAD:HPAD + P] = wh
    rt[10, HPAD:HPAD + P] = wl
    rt[11:13, :] = 1.0

    # pack into 4 partition-group chunks
    wr = np.empty((KROWS, NGRP, CHUNK_W), dtype=np.float16)
    for j in range(NGRP):
        wr[:, j, 0:WT_CW] = wt[:, S * BPG * j:S * BPG * (j + 1)]
        wr[:, j, WT_CW:] = rt[:, (P // NGRP) * j:(P // NGRP) * j + RT_CW]

    return {"wr": wr}, {
        "Xs": Xs.astype(np.float64), "Ys": Ys.astype(np.float64),
        "gap": gap, "hard": hard,
    }


_ORDER_INV = np.argsort(np.array(ORDER))


def _post_core(out, meta):
    """Combine device output into sum over queries of min-D (float64)."""
    inv_s2 = 1.0 / (SCALE * SCALE)
    o = out.astype(np.float64)
    o = np.minimum(o[:, :NOUT], o[:, NOUT:])  # direct-psum vs converted part
    # out col i holds block ORDER[i]
    dmin = o.T[_ORDER_INV].reshape(M) * inv_s2

    # soundness check (device fp16 margin included); hard queries always
    # recomputed exactly on host
    g2 = meta["gap"] * meta["gap"]
    ok = dmin <= g2 - 1e-3 - 2e-3 * np.abs(dmin)
    ok[meta["hard"]] = False
    bad = np.flatnonzero(~ok)
    if len(bad):
        Xb = meta["Xs"][bad]
        Ys = meta["Ys"]
        db = (((Xb[:, None, :] - Ys[None, :, :]) ** 2).sum(-1)).min(axis=1)
        dmin[bad] = db
    return dmin.sum()


def _install_axon_profile_hook():
    """Make trace=True work under axon when the image's antenv lacks
    axon_hooks: inject a shim module wired to the ctypes NTFF driver."""
    import sys
    import types
    try:
        from antenv.axon_hooks import get_axon_ntff_profile_hook  # noqa: F401
        return
    except ImportError:
        pass
    try:
        import antenv
        from trn_agent_boot.trn_boot import _ntff_profile_via_ctypes
        hook = _ntff_profile_via_ctypes("/opt/axon/libaxon_pjrt.so")
    except Exception:
        hook = None
    mod = types.ModuleType("antenv.axon_hooks")
    state = {"h": hook}
    mod.get_axon_ntff_profile_hook = lambda: state["h"]
    mod.set_axon_ntff_profile_hook = lambda h: state.__setitem__("h", h)
    sys.modules["antenv.axon_hooks"] = mod
    try:
        antenv.axon_hooks = mod
    except Exception:
        pass


def kernel(x_hat, points, likelihoods):
    from concourse.bass_utils import run_bass_kernel_spmd
    global LAST_RESULTS

    trace = bool(int(os.environ.get("CHAMFER_TRACE", "0")))
    if trace:
        _install_axon_profile_hook()

    if "nc" not in _CACHE:
        _CACHE["nc"] = _build_bass()
    nc = _CACHE["nc"]

    in_maps, metas = [], []
    for core in range(8):
        b, d = core // 2, core % 2
        X = x_hat[b] if d == 0 else points[b]
        Y = points[b] if d == 0 else x_hat[b]
        m, meta = _prep_core(np.asarray(X), np.asarray(Y))
        in_maps.append(m)
        metas.append(meta)

    res = run_bass_kernel_spmd(
        nc, in_maps, core_ids=list(range(8)), trace=trace,
    )
    LAST_RESULTS = res

    sums = [_post_core(res.results[c]["out"], metas[c]) for c in range(8)]
    cham_x = sum(sums[c] for c in range(8) if c % 2 == 0) / (B * M)
    cham_y = sum(sums[c] for c in range(8) if c % 2 == 1) / (B * P)
    rec = cham_x + cham_y

    lik = np.asarray(likelihoods, dtype=np.float64)
    bpp = np.log2(lik).sum() / (-(B * P))

    loss = bpp + LMBDA * rec
    return np.array([loss, bpp, rec], dtype=np.float32)
